# revision 1
# baseline (speedup 1.0000x reference)
"""Trainium2 Bass kernel for nn_Block_24532853194876 (dense transformer block).

Sharding: data-parallel over batch (64 -> 8 per core) across 8 NeuronCores.

Per-core strategy:
  - tokens flattened to 8*257=2056, padded to 2176=17*128
  - LayerNorms token-major via bn_stats; norm affines folded into the
    following matmul weights on host
  - dense matmuls in fp32r (full PE rate); activations transposed with
    exact fp32 PE transposes
  - attention per (b,h) computed transposed: S^T = k^T x q^T, exp without
    max subtraction (scores ~N(0,1)); softmax normalization via an
    augmented ones-column in V producing row sums in the AV product
  - MLP feature-major; ffn_ln stats via ones-column PE reductions; mean
    removal folded into w3 via mu-row x -colsum(W3); rstd applied on the
    way out
"""
import sys
sys.path.insert(0, '/opt/trn_rl_repo')
import numpy as np

B, NT, D, H, HD, HID = 64, 257, 1024, 16, 64, 4096
NCORES = 8
BL = B // NCORES            # 8 batch elems per core
T = BL * NT                 # 2056 real tokens per core
TP = 2176                   # padded tokens = 17*128
NCH = TP // 128             # 17 token chunks
KD = D // 128               # 8 feature chunks of D
MH = HID // 128             # 32 feature chunks of HID
QP = 260                    # padded per-batch token dim (even, for f32r)
LN_EPS = 1e-5
WS = 16
TOK_SLICES = [(0, 512), (512, 512), (1024, 512), (1536, 384), (1920, 256)]
KCH = [(0, 128), (128, 128), (256, 1)]     # per-b key-token chunks
QCH = [(0, 128), (128, 128), (256, 4)]     # per-b query chunks (padded)

_CACHE = {}


def _rel_pos_index():
    ch, cw = np.meshgrid(np.arange(WS), np.arange(WS), indexing='ij')
    flat = np.stack([ch.reshape(-1), cw.reshape(-1)])
    rel = flat[:, :, None] - flat[:, None, :]
    rel = rel.transpose(1, 2, 0).astype(np.int64)
    rel[:, :, 0] += WS - 1
    rel[:, :, 1] += WS - 1
    rel[:, :, 0] *= 2 * WS - 1
    nrd = (2 * WS - 1) * (2 * WS - 1) + 3
    idx = np.zeros((NT, NT), dtype=np.int64)
    idx[1:, 1:] = rel.sum(-1)
    idx[0, 0:] = nrd - 3
    idx[0:, 0] = nrd - 2
    idx[0, 0] = nrd - 1
    return idx


def _fix_wait_limits(nc, mybir, max_waits=1):
    """This walrus build allows only 1 sync-wait per lowered instruction.
    Hoist excess waits onto same-engine NOPs placed immediately before the
    instruction (engines execute their stream in order)."""
    for f in nc.m.functions:
        for bb in f.blocks:
            insts = bb.instructions
            i = 0
            while i < len(insts):
                inst = insts[i]
                si = inst.sync_info
                if si and si.on_wait and len(si.on_wait) > max_waits:
                    extra = si.on_wait[:-max_waits]
                    inst.sync_info.on_wait = si.on_wait[-max_waits:]
                    for j, w in enumerate(extra):
                        nop = mybir.InstNoOp(
                            name=f"{inst.name}-ws{j}", engine=inst.engine,
                            ins=[], outs=[],
                            sync_info=mybir.SyncInfo(on_wait=[w], on_update=[]),
                        )
                        insts.insert(i, nop)
                        i += 1
                i += 1


def build_module():
    import os
    PH = set((os.environ.get('KPHASES') or 'A,B1,B2,C,D,E0,E,G').split(','))
    DBG = os.environ.get('KDBG') == '1'
    REP = int(os.environ.get('KREPEAT') or '1')
    key = ('nc', tuple(sorted(PH)), DBG, REP)
    if key in _CACHE:
        return _CACHE[key]
    import concourse.bass as bass
    import concourse.mybir as mybir
    import concourse.tile as tile

    F32 = mybir.dt.float32
    F32R = mybir.dt.float32r
    AF = mybir.ActivationFunctionType
    ALU = mybir.AluOpType

    nc = bass.Bass()
    P = lambda name, shape: nc.declare_dram_parameter(name, shape, F32, isOutput=False)

    x_in = P("x", [TP, D])
    xpb = P("xpb", [TP, D])
    wqT = P("wqT", [D, D])
    wkT = P("wkT", [D, D])
    wvT = P("wvT", [D, D])
    wpT = P("wpT", [D, D])
    w1T = P("w1T", [D, HID])
    w2T = P("w2T", [D, HID])
    w3T = P("w3T", [HID, D])
    qb = P("qb", [D, 1])
    kb = P("kb", [D, 1])
    w1b = P("w1b", [HID, 1])
    w2b = P("w2b", [HID, 1])
    vbr = P("vbr", [1, D])
    w3br = P("w3br", [1, D])
    negcs3 = P("negcs3", [1, D])
    cosr = P("cosr", [128, TP])
    sinr = P("sinr", [128, TP])
    r2T = P("r2T", [128, 128])
    identw = P("identw", [128, 128])
    biasT = P("biasT", [H, NT, QP])
    onescol = P("onescol", [128, 1])
    zeros65 = P("zeros65", [128, 65])
    out = nc.declare_dram_parameter("out", [TP, D], F32, isOutput=True)

    if DBG:
        S = lambda name, shape: nc.declare_dram_parameter(name, shape, F32, isOutput=True)
    else:
        S = lambda name, shape: nc.dram_tensor(name, shape, F32)
    qd = S("qd", [D, TP])
    kd = S("kd", [D, TP])
    vd = S("vd", [TP, D])
    od = S("od", [TP, D])
    x2d = S("x2d", [TP, D])
    x2bd = S("x2bd", [TP, D])
    hidd = S("hidd", [HID, TP])
    rstd_sc = S("rstd_sc", [TP, 1])

    with tile.TileContext(nc) as tc:
      for _rep in range(REP):
        with tc.tile_pool(name="consts", bufs=1) as consts:
            ident = consts.tile([128, 128], F32)
            nc.sync.dma_start(out=ident, in_=identw[:, :])
            ones_c = consts.tile([128, 1], F32R)
            nc.sync.dma_start(out=ones_c, in_=onescol[:, :].bitcast(F32R))
            eps_t = consts.tile([128, 1], F32)
            nc.vector.memset(eps_t, LN_EPS)

            def ln_chunk(pool, src, c):
                """DMA chunk c of src [TP, D], return standardized fp32 tile."""
                xt = pool.tile([128, D], F32, tag="ln_x")
                nc.sync.dma_start(out=xt, in_=src[c*128:(c+1)*128, :])
                st = pool.tile([128, 2, 6], F32, tag="ln_st")
                for g in range(2):
                    nc.vector.bn_stats(out=st[:, g, :], in_=xt[:, g*512:(g+1)*512])
                mv = pool.tile([128, 2], F32, tag="ln_mv")
                nc.vector.bn_aggr(out=mv, in_=st)
                sd = pool.tile([128, 1], F32, tag="ln_sd")
                nc.scalar.activation(out=sd, in_=mv[:, 1:2], func=AF.Sqrt,
                                     bias=eps_t, scale=1.0)
                rs = pool.tile([128, 1], F32, tag="ln_rs")
                nc.vector.reciprocal(out=rs, in_=sd)
                xh = pool.tile([128, D], F32, tag="ln_xh")
                nc.vector.tensor_scalar(out=xh, in0=xt, scalar1=mv[:, 0:1],
                                        scalar2=rs, op0=ALU.subtract, op1=ALU.mult)
                return xh

            def transpose_chunk(ppool, xh, dstT, c):
                """xh [128tok, D] fp32 -> dstT[k][:, c-cols] f32r."""
                for k in range(KD):
                    pt = ppool.tile([128, 128], F32, tag="tr_ps")
                    nc.tensor.transpose(pt, xh[:, k*128:(k+1)*128], ident)
                    nc.scalar.activation(out=dstT[k][:, c*128:(c+1)*128], in_=pt,
                                         func=AF.Copy)

            # ================= A + B: norm1, qkv =================
            with tc.tile_pool(name="ra", bufs=1) as ra:
                xhatT = [ra.tile([128, TP], F32R, name=f"xhatT{k}", tag=f"xhatT{k}") for k in range(KD)]
                with tc.tile_pool(name="pa", bufs=3) as pa, \
                     tc.tile_pool(name="psa", bufs=4, space="PSUM") as psa:
                    for c in range(NCH if 'A' in PH else 0):
                        xh = ln_chunk(pa, x_in, c)
                        transpose_chunk(psa, xh, xhatT, c)
                    zt = pa.tile([128, D], F32, tag="zpad")  # pad rows of od
                    nc.vector.memset(zt, 0.0)
                    nc.sync.dma_start(out=od[T:TP, :], in_=zt[:TP-T, :])

                # ---- B1: q,k + rope ----
                with tc.tile_pool(name="pbw", bufs=1) as pbw, \
                     tc.tile_pool(name="pb", bufs=3) as pb, \
                     tc.tile_pool(name="psb", bufs=3, space="PSUM") as psb:
                    wq = [pbw.tile([128, D], F32R, name=f"wq{k}", tag=f"wq{k}") for k in range(KD)]
                    wk = [pbw.tile([128, D], F32R, name=f"wk{k}", tag=f"wk{k}") for k in range(KD)]
                    for k in range(KD):
                        nc.sync.dma_start(out=wq[k], in_=wqT[k*128:(k+1)*128, :].bitcast(F32R))
                        nc.sync.dma_start(out=wk[k], in_=wkT[k*128:(k+1)*128, :].bitcast(F32R))
                    r2 = pbw.tile([128, 128], F32R)
                    nc.sync.dma_start(out=r2, in_=r2T[:, :].bitcast(F32R))
                    cos_t = pbw.tile([128, TP], F32)
                    sin_t = pbw.tile([128, TP], F32)
                    nc.sync.dma_start(out=cos_t, in_=cosr[:, :])
                    nc.sync.dma_start(out=sin_t, in_=sinr[:, :])
                    qb_t = pbw.tile([128, KD], F32)
                    kb_t = pbw.tile([128, KD], F32)
                    for k in range(KD):
                        nc.sync.dma_start(out=qb_t[:, k:k+1], in_=qb[k*128:(k+1)*128, :])
                        nc.sync.dma_start(out=kb_t[:, k:k+1], in_=kb[k*128:(k+1)*128, :])

                    for (toff, tlen) in (TOK_SLICES if 'B1' in PH else []):
                        for m in range(KD):
                            for (wt, bt, dst) in ((wq, qb_t, qd), (wk, kb_t, kd)):
                                ps = psb.tile([128, 512], F32, tag="qk_ps")
                                for k in range(KD):
                                    nc.tensor.matmul(ps[:, :tlen],
                                                     wt[k][:, m*128:(m+1)*128],
                                                     xhatT[k][:, toff:toff+tlen],
                                                     start=(k == 0), stop=(k == KD-1))
                                qr = pb.tile([128, 512], F32R, tag="qk_r")
                                nc.vector.tensor_scalar_add(qr[:, :tlen], ps[:, :tlen],
                                                            bt[:, m:m+1])
                                ps2 = psb.tile([128, 512], F32, tag="qk_ps2")
                                nc.tensor.matmul(ps2[:, :tlen], r2, qr[:, :tlen],
                                                 start=True, stop=True)
                                t1 = pb.tile([128, 512], F32, tag="qk_t1")
                                nc.vector.tensor_mul(t1[:, :tlen], ps2[:, :tlen],
                                                     sin_t[:, toff:toff+tlen])
                                t2 = pb.tile([128, 512], F32, tag="qk_t2")
                                nc.vector.tensor_mul(t2[:, :tlen], qr[:, :tlen],
                                                     cos_t[:, toff:toff+tlen])
                                qf = pb.tile([128, 512], F32R, tag="qk_f")
                                nc.vector.tensor_add(qf[:, :tlen], t1[:, :tlen],
                                                     t2[:, :tlen])
                                nc.sync.dma_start(
                                    out=dst[m*128:(m+1)*128,
                                            toff:toff+tlen].bitcast(F32R),
                                    in_=qf[:, :tlen])

                # ---- B2: v ----
                with tc.tile_pool(name="pvw", bufs=1) as pvw, \
                     tc.tile_pool(name="pv", bufs=3) as pv, \
                     tc.tile_pool(name="psv", bufs=4, space="PSUM") as psv:
                    wv = [pvw.tile([128, D], F32R, name=f"wv{k}", tag=f"wv{k}") for k in range(KD)]
                    for k in range(KD):
                        nc.sync.dma_start(out=wv[k], in_=wvT[k*128:(k+1)*128, :].bitcast(F32R))
                    vb_b = pvw.tile([128, D], F32)
                    nc.sync.dma_start(out=vb_b, in_=vbr[0:1, :].partition_broadcast(128))
                    for c in range(NCH if 'B2' in PH else 0):
                        for ns in range(2):
                            ps = psv.tile([128, 512], F32, tag="v_ps")
                            for k in range(KD):
                                nc.tensor.matmul(ps, xhatT[k][:, c*128:(c+1)*128],
                                                 wv[k][:, ns*512:(ns+1)*512],
                                                 start=(k == 0), stop=(k == KD-1))
                            vt = pv.tile([128, 512], F32R, tag="v_t")
                            nc.vector.tensor_add(vt, ps, vb_b[:, ns*512:(ns+1)*512])
                            nc.sync.dma_start(out=vd[c*128:(c+1)*128,
                                              ns*512:(ns+1)*512].bitcast(F32R), in_=vt)

            # ================= C: attention =================
            with tc.tile_pool(name="pcb", bufs=2) as pcb, \
                 tc.tile_pool(name="pc", bufs=3) as pc, \
                 tc.tile_pool(name="psc", bufs=2, space="PSUM") as psc:
                for h in range(H if 'C' in PH else 0):
                    bts = []
                    for kc, (koff, klen) in enumerate(KCH):
                        bt = pcb.tile([128, QP], F32, tag=f"biasT{kc}")
                        nc.sync.dma_start(out=bt[:klen, :],
                                          in_=biasT[h, koff:koff+klen, :])
                        bts.append(bt)
                    for b in range(BL):
                        t0 = b * NT
                        qt = pc.tile([64, QP], F32R, tag="at_q")
                        nc.sync.dma_start(out=qt, in_=qd[h*64:(h+1)*64,
                                          t0:t0+QP].bitcast(F32R))
                        kt = pc.tile([64, QP], F32R, tag="at_k")
                        nc.sync.dma_start(out=kt, in_=kd[h*64:(h+1)*64,
                                          t0:t0+QP].bitcast(F32R))
                        vts = []
                        for kc, (koff, klen) in enumerate(KCH):
                            vt = pc.tile([128, 65], F32R, tag=f"at_v{kc}")
                            if klen < 128:
                                nc.sync.dma_start(out=vt, in_=zeros65[:, :].bitcast(F32R))
                            nc.sync.dma_start(out=vt[:klen, 0:64],
                                              in_=vd[t0+koff:t0+koff+klen,
                                                     h*64:(h+1)*64].bitcast(F32R))
                            nc.sync.dma_start(out=vt[:klen, 64:65],
                                              in_=onescol[:klen, :].bitcast(F32R))
                            vts.append(vt)
                        pts = []
                        for kc, (koff, klen) in enumerate(KCH):
                            ps = psc.tile([128, QP], F32, tag="at_s")
                            nc.tensor.matmul(ps[:klen, :], kt[:, koff:koff+klen], qt,
                                             start=True, stop=True)
                            sb = pc.tile([128, QP], F32, tag="at_sb")
                            nc.vector.tensor_add(sb[:klen, :], ps[:klen, :],
                                                 bts[kc][:klen, :])
                            pt = pc.tile([128, QP], F32R, tag="at_p")
                            nc.scalar.activation(out=pt[:klen, :], in_=sb[:klen, :],
                                                 func=AF.Exp)
                            pts.append(pt)
                        po = psc.tile([65, QP], F32, tag="at_o")
                        for kc, (koff, klen) in enumerate(KCH):
                            nc.tensor.matmul(po, vts[kc][:klen, :], pts[kc][:klen, :],
                                             start=(kc == 0), stop=(kc == 2))
                        ot = pc.tile([65, QP], F32, tag="at_ot")
                        nc.scalar.activation(out=ot, in_=po, func=AF.Copy)
                        for (qoff, qlen) in QCH:
                            valid = min(qlen, NT - qoff)
                            ptr = psc.tile([128, 65], F32, tag="at_tr")
                            nc.tensor.transpose(ptr[:qlen, :], ot[:, qoff:qoff+qlen],
                                                ident[:65, :65])
                            rec = pc.tile([128, 1], F32, tag="at_rc")
                            nc.vector.reciprocal(rec[:valid], ptr[:valid, 64:65])
                            oo = pc.tile([128, 64], F32, tag="at_oo")
                            nc.vector.tensor_scalar_mul(oo[:valid], ptr[:valid, 0:64],
                                                        rec[:valid])
                            nc.sync.dma_start(
                                out=od[t0+qoff:t0+qoff+valid, h*64:(h+1)*64],
                                in_=oo[:valid])

            # ========== D: inner LN -> oT; proj -> x2, x2b ==========
            with tc.tile_pool(name="rd", bufs=1) as rd:
                oT = [rd.tile([128, TP], F32R, name=f"oT{k}", tag=f"oT{k}") for k in range(KD)]
                with tc.tile_pool(name="pd", bufs=3) as pd, \
                     tc.tile_pool(name="psd", bufs=4, space="PSUM") as psd:
                    for c in range(NCH if 'D' in PH else 0):
                        xh = ln_chunk(pd, od, c)
                        transpose_chunk(psd, xh, oT, c)
                with tc.tile_pool(name="pdw", bufs=1) as pdw, \
                     tc.tile_pool(name="pd2", bufs=3) as pd2, \
                     tc.tile_pool(name="psd2", bufs=4, space="PSUM") as psd2:
                    wp = [pdw.tile([128, D], F32R, name=f"wp{k}", tag=f"wp{k}") for k in range(KD)]
                    for k in range(KD):
                        nc.sync.dma_start(out=wp[k], in_=wpT[k*128:(k+1)*128, :].bitcast(F32R))
                    w3b_b = pdw.tile([128, D], F32)
                    nc.sync.dma_start(out=w3b_b, in_=w3br[0:1, :].partition_broadcast(128))
                    for c in range(NCH if 'D' in PH else 0):
                        for ns in range(2):
                            ps = psd2.tile([128, 512], F32, tag="pj_ps")
                            for k in range(KD):
                                nc.tensor.matmul(ps, oT[k][:, c*128:(c+1)*128],
                                                 wp[k][:, ns*512:(ns+1)*512],
                                                 start=(k == 0), stop=(k == KD-1))
                            xr = pd2.tile([128, 512], F32, tag="pj_xr")
                            nc.sync.dma_start(out=xr, in_=xpb[c*128:(c+1)*128,
                                                            ns*512:(ns+1)*512])
                            x2t = pd2.tile([128, 512], F32, tag="pj_x2")
                            nc.vector.tensor_add(x2t, ps, xr)
                            nc.sync.dma_start(out=x2d[c*128:(c+1)*128,
                                              ns*512:(ns+1)*512], in_=x2t)
                            x2bt = pd2.tile([128, 512], F32, tag="pj_x2b")
                            nc.vector.tensor_add(x2bt, x2t,
                                                 w3b_b[:, ns*512:(ns+1)*512])
                            nc.sync.dma_start(out=x2bd[c*128:(c+1)*128,
                                              ns*512:(ns+1)*512], in_=x2bt)

            # ========== E0/E/G: norm2, MLP, w3 ==========
            with tc.tile_pool(name="pmur", bufs=1) as pmur:
                mur = pmur.tile([1, TP], F32R)

                with tc.tile_pool(name="re", bufs=1) as rep:
                    x2T = [rep.tile([128, TP], F32R, name=f"x2T{k}", tag=f"x2T{k}") for k in range(KD)]
                    with tc.tile_pool(name="pe0", bufs=3) as pe0, \
                         tc.tile_pool(name="pse0", bufs=4, space="PSUM") as pse0:
                        for c in range(NCH if 'E0' in PH else 0):
                            xh = ln_chunk(pe0, x2d, c)
                            transpose_chunk(pse0, xh, x2T, c)

                    with tc.tile_pool(name="pst", bufs=1) as pst, \
                         tc.tile_pool(name="pew", bufs=2) as pew, \
                         tc.tile_pool(name="pe", bufs=3) as pe, \
                         tc.tile_pool(name="pse", bufs=2, space="PSUM") as pse:
                        sx = pst.tile([1, TP], F32)
                        sxx = pst.tile([1, TP], F32)
                        nc.vector.memset(sx, 0.0)
                        nc.vector.memset(sxx, 0.0)
                        rsr = pst.tile([1, TP], F32)
                        w1b_t = pst.tile([128, MH], F32)
                        w2b_t = pst.tile([128, MH], F32)
                        for m in range(MH):
                            nc.sync.dma_start(out=w1b_t[:, m:m+1],
                                              in_=w1b[m*128:(m+1)*128, :])
                            nc.sync.dma_start(out=w2b_t[:, m:m+1],
                                              in_=w2b[m*128:(m+1)*128, :])
                        for m in range(MH if 'E' in PH else 0):
                            w1m = [pew.tile([128, 128], F32R, name=f"w1m{k}", tag=f"w1m{k}")
                                   for k in range(KD)]
                            w2m = [pew.tile([128, 128], F32R, name=f"w2m{k}", tag=f"w2m{k}")
                                   for k in range(KD)]
                            for k in range(KD):
                                nc.sync.dma_start(
                                    out=w1m[k], in_=w1T[k*128:(k+1)*128,
                                    m*128:(m+1)*128].bitcast(F32R))
                                nc.sync.dma_start(
                                    out=w2m[k], in_=w2T[k*128:(k+1)*128,
                                    m*128:(m+1)*128].bitcast(F32R))
                            for (toff, tlen) in TOK_SLICES:
                                ps1 = pse.tile([128, 512], F32, tag="e_ps1")
                                for k in range(KD):
                                    nc.tensor.matmul(ps1[:, :tlen], w1m[k],
                                                     x2T[k][:, toff:toff+tlen],
                                                     start=(k == 0), stop=(k == KD-1))
                                h1t = pe.tile([128, 512], F32, tag="e_h1")
                                nc.scalar.activation(out=h1t[:, :tlen], in_=ps1[:, :tlen],
                                                     func=AF.Silu,
                                                     bias=w1b_t[:, m:m+1], scale=1.0)
                                ps2 = pse.tile([128, 512], F32, tag="e_ps2")
                                for k in range(KD):
                                    nc.tensor.matmul(ps2[:, :tlen], w2m[k],
                                                     x2T[k][:, toff:toff+tlen],
                                                     start=(k == 0), stop=(k == KD-1))
                                h2t = pe.tile([128, 512], F32, tag="e_h2")
                                nc.vector.tensor_scalar_add(h2t[:, :tlen], ps2[:, :tlen],
                                                            w2b_t[:, m:m+1])
                                ht = pe.tile([128, 512], F32R, tag="e_h")
                                nc.vector.tensor_mul(ht[:, :tlen], h1t[:, :tlen],
                                                     h2t[:, :tlen])
                                hq = pe.tile([128, 512], F32R, tag="e_hq")
                                nc.vector.tensor_mul(hq[:, :tlen], ht[:, :tlen],
                                                     ht[:, :tlen])
                                nc.sync.dma_start(out=hidd[m*128:(m+1)*128,
                                                  toff:toff+tlen].bitcast(F32R),
                                                  in_=ht[:, :tlen])
                                pssx = pse.tile([1, 512], F32, tag="e_sx")
                                nc.tensor.matmul(pssx[:, :tlen], ones_c, ht[:, :tlen],
                                                 start=True, stop=True)
                                nc.vector.tensor_add(sx[:, toff:toff+tlen],
                                                     sx[:, toff:toff+tlen],
                                                     pssx[:, :tlen])
                                pssq = pse.tile([1, 512], F32, tag="e_sq")
                                nc.tensor.matmul(pssq[:, :tlen], ones_c, hq[:, :tlen],
                                                 start=True, stop=True)
                                nc.vector.tensor_add(sxx[:, toff:toff+tlen],
                                                     sxx[:, toff:toff+tlen],
                                                     pssq[:, :tlen])
                        # ffn stats
                        nc.scalar.activation(out=mur, in_=sx, func=AF.Copy,
                                             scale=1.0/HID)
                        ex2 = pst.tile([1, TP], F32, tag="e_ex2")
                        nc.scalar.activation(out=ex2, in_=sxx, func=AF.Copy,
                                             scale=1.0/HID)
                        mu2 = pst.tile([1, TP], F32, tag="e_mu2")
                        nc.vector.tensor_mul(mu2, mur, mur)
                        varr = pst.tile([1, TP], F32, tag="e_var")
                        nc.vector.tensor_sub(varr, ex2, mu2)
                        sdr = pst.tile([1, TP], F32, tag="e_sd")
                        nc.scalar.activation(out=sdr, in_=varr, func=AF.Sqrt,
                                             bias=eps_t[0:1, :], scale=1.0)
                        nc.vector.reciprocal(rsr, sdr)
                        nc.sync.dma_start(out=rstd_sc[:, 0:1], in_=rsr[0:1, :])

                # ---- G: w3 ----
                with tc.tile_pool(name="pgw", bufs=1) as pgw, \
                     tc.tile_pool(name="pgh", bufs=2) as pgh, \
                     tc.tile_pool(name="pg", bufs=3) as pg, \
                     tc.tile_pool(name="psg", bufs=4, space="PSUM") as psg:
                    w3 = [pgw.tile([128, D], F32R, name=f"w3_{m}", tag=f"w3_{m}") for m in range(MH)]
                    for m in range(MH):
                        nc.sync.dma_start(out=w3[m],
                                          in_=w3T[m*128:(m+1)*128, :].bitcast(F32R))
                    ncs = pgw.tile([1, D], F32R)
                    nc.sync.dma_start(out=ncs, in_=negcs3[:, :].bitcast(F32R))
                    for c in range(NCH if 'G' in PH else 0):
                        hblk = []
                        for m in range(MH):
                            hb = pgh.tile([128, 128], F32R, tag=f"g_h{m}")
                            nc.sync.dma_start(out=hb, in_=hidd[m*128:(m+1)*128,
                                              c*128:(c+1)*128].bitcast(F32R))
                            hblk.append(hb)
                        rsc = pg.tile([128, 1], F32, tag="g_rs")
                        nc.sync.dma_start(out=rsc, in_=rstd_sc[c*128:(c+1)*128, :])
                        x2bt = pg.tile([128, D], F32, tag="g_x2b")
                        nc.sync.dma_start(out=x2bt, in_=x2bd[c*128:(c+1)*128, :])
                        for ns in range(2):
                            ps = psg.tile([128, 512], F32, tag="g_ps")
                            for m in range(MH):
                                nc.tensor.matmul(ps, hblk[m],
                                                 w3[m][:, ns*512:(ns+1)*512],
                                                 start=(m == 0), stop=False)
                            nc.tensor.matmul(ps, mur[:, c*128:(c+1)*128],
                                             ncs[:, ns*512:(ns+1)*512],
                                             start=False, stop=True)
                            ut = pg.tile([128, 512], F32, tag="g_u")
                            nc.vector.tensor_scalar_mul(ut, ps, rsc)
                            ot2 = pg.tile([128, 512], F32, tag="g_o")
                            nc.vector.tensor_add(ot2, ut, x2bt[:, ns*512:(ns+1)*512])
                            nc.sync.dma_start(out=out[c*128:(c+1)*128,
                                              ns*512:(ns+1)*512], in_=ot2)

    _fix_wait_limits(nc, mybir)
    _CACHE[key] = nc
    return nc


def prep_inputs(inputs):
    f32 = np.float32
    x = np.asarray(inputs['x'], f32)
    g1 = np.asarray(inputs['norm1_g'], f32); b1 = np.asarray(inputs['norm1_b'], f32)
    gi = np.asarray(inputs['inner_ln_g'], f32); bi = np.asarray(inputs['inner_ln_b'], f32)
    g2 = np.asarray(inputs['norm2_g'], f32); b2 = np.asarray(inputs['norm2_b'], f32)
    gf = np.asarray(inputs['ffn_ln_g'], f32); bf = np.asarray(inputs['ffn_ln_b'], f32)
    q_w = np.asarray(inputs['q_w'], f32); q_b = np.asarray(inputs['q_b'], f32)
    k_w = np.asarray(inputs['k_w'], f32)
    v_w = np.asarray(inputs['v_w'], f32); v_b = np.asarray(inputs['v_b'], f32)
    p_w = np.asarray(inputs['proj_w'], f32); p_b = np.asarray(inputs['proj_b'], f32)
    w1_w = np.asarray(inputs['w1_w'], f32); w1_b = np.asarray(inputs['w1_b'], f32)
    w2_w = np.asarray(inputs['w2_w'], f32); w2_b = np.asarray(inputs['w2_b'], f32)
    w3_w = np.asarray(inputs['w3_w'], f32); w3_b = np.asarray(inputs['w3_b'], f32)
    tab = np.asarray(inputs['rel_bias_table'], f32)
    rc = np.asarray(inputs['rope_cos'], f32)
    rs = np.asarray(inputs['rope_sin'], f32)

    scale = HD ** -0.5
    wqTe = (q_w.T * g1[:, None] * scale).astype(f32)
    qb_eff = ((q_b + q_w @ b1) * scale).astype(f32)
    wkTe = (k_w.T * g1[:, None]).astype(f32)
    kb_eff = (k_w @ b1).astype(f32)
    wvTe = (v_w.T * g1[:, None]).astype(f32)
    vb_eff = (v_b + v_w @ b1).astype(f32)
    wpTe = (p_w.T * gi[:, None]).astype(f32)
    pb_eff = (p_b + p_w @ bi).astype(f32)
    w1Te = (w1_w.T * g2[:, None]).astype(f32)
    w1b_eff = (w1_b + w1_w @ b2).astype(f32)
    w2Te = (w2_w.T * g2[:, None]).astype(f32)
    w2b_eff = (w2_b + w2_w @ b2).astype(f32)
    w3Te = (w3_w.T * gf[:, None]).astype(f32)
    w3b_eff = (w3_b + w3_w @ bf).astype(f32)
    negcs3v = (-w3Te.sum(0, keepdims=True)).astype(f32)

    cosr = np.ones((128, TP), f32)
    sinr = np.zeros((128, TP), f32)
    for b in range(BL):
        cosr[0:64, b*NT+1:(b+1)*NT] = rc.T
        cosr[64:128, b*NT+1:(b+1)*NT] = rc.T
        sinr[0:64, b*NT+1:(b+1)*NT] = rs.T
        sinr[64:128, b*NT+1:(b+1)*NT] = rs.T

    r2 = np.zeros((64, 64), f32)
    for i in range(32):
        r2[2*i, 2*i+1] = -1.0
        r2[2*i+1, 2*i] = 1.0
    r2b = np.zeros((128, 128), f32)
    r2b[0:64, 0:64] = r2
    r2b[64:128, 64:128] = r2

    idx = _rel_pos_index()
    biasTv = np.zeros((H, NT, QP), f32)
    biasTv[:, :, :NT] = tab[idx, :].transpose(2, 1, 0)

    common = {
        'wqT': wqTe, 'wkT': wkTe, 'wvT': wvTe, 'wpT': wpTe,
        'w1T': w1Te, 'w2T': w2Te, 'w3T': w3Te,
        'qb': qb_eff.reshape(D, 1), 'kb': kb_eff.reshape(D, 1),
        'w1b': w1b_eff.reshape(HID, 1), 'w2b': w2b_eff.reshape(HID, 1),
        'vbr': vb_eff.reshape(1, D), 'w3br': w3b_eff.reshape(1, D),
        'negcs3': negcs3v, 'cosr': cosr, 'sinr': sinr, 'r2T': r2b.T.copy(),
        'identw': np.eye(128, dtype=f32), 'biasT': biasTv,
        'onescol': np.ones((128, 1), f32), 'zeros65': np.zeros((128, 65), f32),
    }
    in_maps = []
    for c in range(NCORES):
        xc = np.zeros((TP, D), f32)
        xc[:T] = x[c*BL:(c+1)*BL].reshape(T, D)
        xpbc = xc.copy()
        xpbc[:T] += pb_eff[None, :]
        im = dict(common)
        im['x'] = xc
        im['xpb'] = xpbc
        in_maps.append(im)
    return in_maps


def kernel(**inputs):
    from concourse.bass_utils import run_bass_kernel_spmd
    nc = build_module()
    in_maps = prep_inputs(inputs)
    res = run_bass_kernel_spmd(nc, in_maps, list(range(NCORES)))
    outs = []
    for c in range(NCORES):
        outs.append(res.results[c]['out'][:T].reshape(BL, NT, D))
    return np.concatenate(outs, 0).astype(np.float32)

